# revision 28
# baseline (speedup 1.0000x reference)
"""DRAE loss kernel for Trainium2, 8 NeuronCores (SPMD).

Problem: input/target [8192, 4096] f32.
  Err[n] = sum_d (input[n,d] - target[n,d])^2            (memory-bound part)
  sErr = sort(Err); cs = cumsum(sErr)
  obj(k) = (total2 - cs_k^2/k - (total-cs_k)^2/(N-k)) / Sb   for k=1..N-1
  i = argmin(obj)  (first min);  out = cs[i]/(i+1) + 0.1*obj[i]

Sharding: data-parallel over N across 8 cores (1024 rows each). Per-core
row reductions produce Err_local[1024]; AllGather -> Err[8192]; the sort +
cumsum + argmin + final scalar run replicated on every core on-chip
(exact bitonic sort network on a [128,64]/[64,128] two-layout scheme).

Self-contained: hardcodes shapes; only needs concourse (bass) + numpy.
"""
import numpy as np

import concourse.bass as bass
import concourse.bacc as bacc
import concourse.mybir as mybir
import concourse.tile as tile
from concourse.bass_utils import run_bass_kernel_spmd

F32 = mybir.dt.float32
I32 = mybir.dt.int32

NCORES = 8
N, D = 8192, 4096
ROWS = N // NCORES           # 1024 rows per core
RT = ROWS // 128             # 8 row-tiles of [128, D] per core
W_A, W_B = 64, 128           # layout A: [128, 64]; layout B: [64, 128]
LAMB = 0.1
BIG = np.float32(1e30)

_CACHE = {}


def _emit_sort(nc, ps, ta, tb, tflip, ident):
    """Normalized bitonic sort network (reversal substages, all-ascending).

    ta = [ta0, ta1] ping-pong [128,64] SBUF tiles (layout A: i = 64p + f),
    tb = [tb0, tb1] [64,128] tiles (layout B: i = 64r + q, transpose of A),
    tflip = [128,64] scratch for the free-reversed copy feeding pb_rev,
    ident = [128,128] f32 identity for PE transposes.
    Data starts in ta[0]; returns index of the ta tile holding sorted data.
    """
    mm = mybir.AluOpType
    ia = 0
    ib = 0

    def plain(x_ap, y_ap, b):
        # compare-exchange pairs at stride b in the free dim; lower keeps min
        xv = x_ap.rearrange("p (a t b) -> p a t b", t=2, b=b)
        yv = y_ap.rearrange("p (a t b) -> p a t b", t=2, b=b)
        nc.vector.tensor_tensor(yv[:, :, 0, :], xv[:, :, 0, :], xv[:, :, 1, :], mm.min)
        nc.vector.tensor_tensor(yv[:, :, 1, :], xv[:, :, 0, :], xv[:, :, 1, :], mm.max)

    for s in range(1, 14):
        if s <= 6:
            # reversal substage within the free dim of A
            blk = 1 << s
            h = blk // 2
            x, y = ta[ia][:], ta[1 - ia][:]
            xv = x.rearrange("p (a b) -> p a b", b=blk)
            xr = xv[:, :, ::-1]
            yv = y.rearrange("p (a b) -> p a b", b=blk)
            nc.vector.tensor_tensor(yv[:, :, :h], xv[:, :, :h], xr[:, :, :h], mm.min)
            nc.vector.tensor_tensor(yv[:, :, h:], xv[:, :, h:], xr[:, :, h:], mm.max)
            ia = 1 - ia
            for j in range(s - 2, -1, -1):
                plain(ta[ia][:], ta[1 - ia][:], 1 << j)
                ia = 1 - ia
        else:
            # A -> B with both plain and partition-reversed copies
            nc.vector.tensor_copy(tflip[:], ta[ia][:][:, ::-1])
            pb = ps.tile([64, 128], F32, tag="pb", name="pb")
            nc.tensor.transpose(pb[:], ta[ia][:], ident[:])
            pbr = ps.tile([64, 128], F32, tag="pbr", name="pbr")
            nc.tensor.transpose(pbr[:], tflip[:], ident[:])
            # only one DVE input may come from PSUM: land pb in SBUF first
            nc.vector.tensor_copy(tb[ib][:], pb[:])
            # reversal substage in B: partner(q, r) = (63-q, blockrev(r))
            rblk = 1 << (s - 6)
            h = rblk // 2
            vb = tb[ib][:].rearrange("q (a b) -> q a b", b=rblk)
            vr = pbr[:].rearrange("q (a b) -> q a b", b=rblk)[:, :, ::-1]
            yv = tb[1 - ib][:].rearrange("q (a b) -> q a b", b=rblk)
            nc.vector.tensor_tensor(yv[:, :, :h], vb[:, :, :h], vr[:, :, :h], mm.min)
            nc.vector.tensor_tensor(yv[:, :, h:], vb[:, :, h:], vr[:, :, h:], mm.max)
            ib = 1 - ib
            for j in range(s - 2, 5, -1):
                plain(tb[ib][:], tb[1 - ib][:], 1 << (j - 6))
                ib = 1 - ib
            # B -> A, landing in SBUF before the A substages
            pa = ps.tile([128, 64], F32, tag="psA", name="pa")
            nc.tensor.transpose(pa[:], tb[ib][:], ident[:64, :64])
            nc.vector.tensor_copy(ta[1 - ia][:], pa[:])
            ia = 1 - ia
            for j in range(5, -1, -1):
                plain(ta[ia][:], ta[1 - ia][:], 1 << j)
                ia = 1 - ia
    return ia


def _build(phase2_only=False, stop="full", timing_variant=False):
    ncores = 1 if (phase2_only or timing_variant) else NCORES
    nc = bacc.Bacc("TRN2", target_bir_lowering=False, debug=False,
                   num_devices=ncores)

    if phase2_only:
        err_in = nc.dram_tensor("err", [N], F32, kind="ExternalInput").ap()
        dbg_srt = nc.dram_tensor("dbg_srt", [128, W_A], F32, kind="ExternalOutput").ap()
        dbg_cs = nc.dram_tensor("dbg_cs", [128, W_A], F32, kind="ExternalOutput").ap()
        dbg_obj = nc.dram_tensor("dbg_obj", [128, W_A], F32, kind="ExternalOutput").ap()
    else:
        inp = nc.dram_tensor("input", [ROWS, D], F32, kind="ExternalInput").ap()
        tgt = nc.dram_tensor("target", [ROWS, D], F32, kind="ExternalInput").ap()
    out = nc.dram_tensor("out", [1, 1], F32, kind="ExternalOutput").ap()

    # compile-time constants
    kvals = (np.arange(1, N + 1, dtype=np.float64).reshape(128, 64)).astype(np.float32)
    recip_k = (1.0 / kvals).astype(np.float32)
    nk = (N - kvals).astype(np.float32)
    nk[127, 63] = 1.0  # k = N slot excluded later; avoid 1/0
    recip_nk = (1.0 / nk).astype(np.float32)
    recip_nk[127, 63] = 0.0
    ident_np = np.eye(128, dtype=np.float32)
    triu_np = np.triu(np.ones((128, 128), np.float32), 1)  # [p',p]=1 iff p'<p
    ones_np = np.ones((128, 128), np.float32)
    excl_np = np.zeros((128, 64), np.float32)
    excl_np[127, 63] = BIG  # push k=N out of the argmin

    c_k = nc.inline_tensor(kvals, name="c_k")
    c_rk = nc.inline_tensor(recip_k, name="c_rk")
    c_rnk = nc.inline_tensor(recip_nk, name="c_rnk")
    c_id = nc.inline_tensor(ident_np, name="c_id")
    c_tu = nc.inline_tensor(triu_np, name="c_tu")
    c_on = nc.inline_tensor(ones_np, name="c_on")
    c_ex = nc.inline_tensor(excl_np, name="c_ex")

    mm = mybir.AluOpType
    AF = mybir.ActivationFunctionType

    with tile.TileContext(nc) as tc:
        with (
            tc.tile_pool(name="io", bufs=3) as io,
            tc.tile_pool(name="wk", bufs=2) as wk,
            tc.tile_pool(name="st", bufs=1) as st,
            tc.tile_pool(name="ps", bufs=2, space="PSUM") as pspool,
            tc.tile_pool(name="dram", bufs=1, space="DRAM") as dram,
        ):
            def _body():
                if not phase2_only:
                    # ---------------- phase 1: Err_local ----------------
                    errcol = st.tile([128, RT], F32, name="errcol")
                    for t in range(RT):
                        a = io.tile([128, D], F32, tag="a", name="a")
                        b = io.tile([128, D], F32, tag="b", name="b")
                        nc.sync.dma_start(a[:], inp[t * 128:(t + 1) * 128, :])
                        nc.sync.dma_start(b[:], tgt[t * 128:(t + 1) * 128, :])
                        d = wk.tile([128, D], F32, tag="d", name="d")
                        nc.vector.tensor_tensor(d[:], a[:], b[:], mm.subtract)
                        sq = wk.tile([128, D], F32, tag="sq", name="sq")
                        nc.scalar.activation(sq[:], d[:], AF.Square,
                                             accum_out=errcol[:, t:t + 1])

                    # ---------------- allgather Err ----------------
                    gin = dram.tile([ROWS], F32, name="gin")
                    gout = dram.tile([N], F32, name="gout")
                    nc.sync.dma_start(gin[:].rearrange("(p t) -> p t", t=RT),
                                      errcol[:])
                    if timing_variant:
                        # stand-in for the AllGather: 8 local 4KB DMAs
                        for c in range(NCORES):
                            nc.sync.dma_start(gout[c * ROWS:(c + 1) * ROWS],
                                              gin[:])
                    else:
                        nc.gpsimd.collective_compute(
                            "AllGather", mm.bypass,
                            replica_groups=[list(range(NCORES))],
                            ins=[gin[:]], outs=[gout[:]],
                        )
                    err_src = gout[:]
                    if stop == "phase1":
                        nc.sync.dma_start(out[:], errcol[:1, :1])
                        return
                else:
                    err_src = err_in

                # ---------------- phase 2 (replicated) ----------------
                ta = [st.tile([128, W_A], F32, tag=f"ta{i}", name=f"ta{i}")
                      for i in range(2)]
                tb = [st.tile([64, W_B], F32, tag=f"tb{i}", name=f"tb{i}")
                      for i in range(2)]
                tflip = st.tile([128, W_A], F32, name="tflip")
                ident = st.tile([128, 128], F32, name="ident")
                triu = st.tile([128, 128], F32, name="triu")
                ones = st.tile([128, 128], F32, name="ones")
                kf = st.tile([128, W_A], F32, name="kf")
                rk = st.tile([128, W_A], F32, name="rk")
                rnk = st.tile([128, W_A], F32, name="rnk")
                excl = st.tile([128, W_A], F32, name="excl")
                for tl, cc in ((ident, c_id), (triu, c_tu),
                               (ones, c_on), (kf, c_k), (rk, c_rk),
                               (rnk, c_rnk), (excl, c_ex)):
                    nc.sync.dma_start(tl[:], cc.ap())

                nc.sync.dma_start(ta[0][:],
                                  err_src.rearrange("(p f) -> p f", f=W_A))

                isorted = _emit_sort(nc, pspool, ta, tb, tflip, ident)
                srt = ta[isorted][:]      # sorted ascending, [p,f] = s[64p+f]

                if phase2_only and stop == "sort":
                    nc.sync.dma_start(dbg_srt[:], srt)
                    nc.sync.dma_start(out[:], srt[:1, :1])
                    return

                # row sums of squares -> total2
                sqd = st.tile([128, W_A], F32, name="sqd")
                rowsq = st.tile([128, 1], F32, name="rowsq")
                nc.scalar.activation(sqd[:], srt, AF.Square, accum_out=rowsq[:])

                # in-row inclusive prefix sums (Hillis-Steele)
                cs = [st.tile([128, W_A], F32, tag=f"cs{i}", name=f"cs{i}")
                      for i in range(2)]
                cur, nxt = 0, 1
                nc.vector.tensor_copy(cs[cur][:], srt)
                for sh in (1, 2, 4, 8, 16, 32):
                    nc.vector.tensor_tensor(cs[nxt][:, sh:], cs[cur][:, sh:],
                                            cs[cur][:, :W_A - sh], mm.add)
                    nc.vector.tensor_copy(cs[nxt][:, :sh], cs[cur][:, :sh])
                    cur, nxt = nxt, cur
                rowpref = cs[cur]         # [128,64] within-row inclusive prefix

                # partition-level exclusive prefix + totals via PE
                rowtot = rowpref[:, W_A - 1:W_A]
                pexc = pspool.tile([128, 1], F32, tag="psv", name="pexc")
                nc.tensor.matmul(pexc[:], triu[:], rowtot)
                ptot = pspool.tile([128, 1], F32, tag="psv", name="ptot")
                nc.tensor.matmul(ptot[:], ones[:], rowtot)
                ptot2 = pspool.tile([128, 1], F32, tag="psv", name="ptot2")
                nc.tensor.matmul(ptot2[:], ones[:], rowsq[:])
                exc = st.tile([128, 1], F32, name="exc")
                tot = st.tile([128, 1], F32, name="tot")
                tot2 = st.tile([128, 1], F32, name="tot2")
                nc.vector.tensor_copy(exc[:], pexc[:])
                nc.vector.tensor_copy(tot[:], ptot[:])
                nc.vector.tensor_copy(tot2[:], ptot2[:])

                csf = cs[nxt]             # reuse the other buffer
                nc.vector.tensor_scalar(csf[:], rowpref[:], exc[:], None, mm.add)

                if phase2_only and stop == "cs":
                    nc.sync.dma_start(dbg_cs[:], csf[:])
                    nc.sync.dma_start(out[:], csf[:1, :1])
                    return

                # obj = (total2 - cs^2/k - (tot-cs)^2/(N-k)) / Sb, computed as
                # v/negSb with v = w - total2, negSb = tot*allMean - total2
                t1 = st.tile([128, W_A], F32, tag="t1", name="t1")
                nc.vector.tensor_tensor(t1[:], csf[:], csf[:], mm.mult)
                nc.vector.tensor_tensor(t1[:], t1[:], rk[:], mm.mult)
                u = st.tile([128, W_A], F32, tag="u", name="u")
                nc.vector.tensor_scalar(u[:], csf[:], tot[:], None, mm.subtract)
                nc.vector.tensor_tensor(u[:], u[:], u[:], mm.mult)
                nc.vector.tensor_tensor(u[:], u[:], rnk[:], mm.mult)
                obj = st.tile([128, W_A], F32, tag="obj", name="obj")
                nc.vector.tensor_tensor(obj[:], t1[:], u[:], mm.add)
                nc.vector.tensor_scalar(obj[:], obj[:], tot2[:], None, mm.subtract)

                am = st.tile([128, 1], F32, name="am")   # allMean
                nc.vector.tensor_scalar(am[:], tot[:], float(1.0 / N), None, mm.mult)
                nsb = st.tile([128, 1], F32, name="nsb")  # negSb
                nc.vector.tensor_tensor(nsb[:], tot[:], am[:], mm.mult)
                nc.vector.tensor_tensor(nsb[:], nsb[:], tot2[:], mm.subtract)
                rnsb = st.tile([128, 1], F32, name="rnsb")
                nc.vector.reciprocal(rnsb[:], nsb[:])
                nc.vector.tensor_scalar(obj[:], obj[:], rnsb[:], None, mm.mult)

                # exclude k = N (BIG at the last slot, 0 elsewhere)
                nc.vector.tensor_tensor(obj[:], obj[:], excl[:], mm.add)

                if phase2_only and stop == "obj":
                    nc.sync.dma_start(dbg_obj[:], obj[:])
                    nc.sync.dma_start(out[:], obj[:1, :1])
                    return

                # argmin (first-min): gmin, then smallest k with obj==gmin
                rmin = st.tile([128, 1], F32, name="rmin")
                nc.vector.tensor_reduce(rmin[:], obj[:], mybir.AxisListType.X, mm.min)
                prm = pspool.tile([1, 128], F32, tag="psv", name="prm")
                nc.tensor.transpose(prm[:], rmin[:], ident[:])
                gmin = st.tile([1, 1], F32, name="gmin")
                nc.vector.tensor_reduce(gmin[:], prm[:], mybir.AxisListType.X, mm.min)
                pgm = pspool.tile([128, 1], F32, tag="psv", name="pgm")
                nc.tensor.matmul(pgm[:], ones[:1, :], gmin[:])
                gminb = st.tile([128, 1], F32, name="gminb")
                nc.vector.tensor_copy(gminb[:], pgm[:])

                eq = st.tile([128, W_A], I32, tag="eq", name="eq")
                nc.vector.tensor_scalar(eq[:], obj[:], gminb[:], None, mm.is_equal)
                idxv = st.tile([128, W_A], F32, tag="idxv", name="idxv")
                nc.vector.memset(idxv[:], float(BIG))
                nc.vector.copy_predicated(idxv[:], eq[:], kf[:])
                ridx = st.tile([128, 1], F32, name="ridx")
                nc.vector.tensor_reduce(ridx[:], idxv[:], mybir.AxisListType.X, mm.min)
                pri = pspool.tile([1, 128], F32, tag="psv", name="pri")
                nc.tensor.transpose(pri[:], ridx[:], ident[:])
                gidx = st.tile([1, 1], F32, name="gidx")
                nc.vector.tensor_reduce(gidx[:], pri[:], mybir.AxisListType.X, mm.min)
                pgi = pspool.tile([128, 1], F32, tag="psv", name="pgi")
                nc.tensor.matmul(pgi[:], ones[:1, :], gidx[:])
                gidxb = st.tile([128, 1], F32, name="gidxb")
                nc.vector.tensor_copy(gidxb[:], pgi[:])

                if phase2_only and stop == "argmin":
                    nc.sync.dma_start(out[:], gidx[:])
                    return

                # cs[i*] via one-hot dot
                oh = st.tile([128, W_A], F32, tag="oh", name="oh")
                nc.vector.tensor_scalar(oh[:], kf[:], gidxb[:], None, mm.is_equal)
                dump = st.tile([128, W_A], F32, tag="dump", name="dump")
                csrow = st.tile([128, 1], F32, name="csrow")
                nc.vector.tensor_tensor(dump[:], csf[:], oh[:], mm.mult)
                nc.vector.tensor_reduce(csrow[:], dump[:], mybir.AxisListType.X,
                                        mm.add)
                pcr = pspool.tile([1, 128], F32, tag="psv", name="pcr")
                nc.tensor.transpose(pcr[:], csrow[:], ident[:])
                cssum = st.tile([1, 1], F32, name="cssum")
                nc.vector.tensor_reduce(cssum[:], pcr[:], mybir.AxisListType.X, mm.add)

                # out = cssum/T + 0.1*gmin
                rT = st.tile([1, 1], F32, name="rT")
                nc.vector.reciprocal(rT[:], gidx[:])
                res = st.tile([1, 1], F32, name="res")
                nc.vector.tensor_tensor(res[:], cssum[:], rT[:], mm.mult)
                sg = st.tile([1, 1], F32, name="sg")
                nc.vector.tensor_scalar(sg[:], gmin[:], LAMB, None, mm.mult)
                nc.vector.tensor_tensor(res[:], res[:], sg[:], mm.add)
                nc.sync.dma_start(out[:], res[:])

                if phase2_only:
                    nc.sync.dma_start(dbg_srt[:], srt)
                    nc.sync.dma_start(dbg_cs[:], csf[:])
                    nc.sync.dma_start(dbg_obj[:], obj[:])

            _body()

    nc.compile()
    return nc


def _get_program():
    if "nc" not in _CACHE:
        _CACHE["nc"] = _build()
    return _CACHE["nc"]


def _run(input, target, trace=False):
    nc = _get_program()
    input = np.ascontiguousarray(input, dtype=np.float32)
    target = np.ascontiguousarray(target, dtype=np.float32)
    assert input.shape == (N, D) and target.shape == (N, D)
    in_maps = [
        {"input": input[c * ROWS:(c + 1) * ROWS],
         "target": target[c * ROWS:(c + 1) * ROWS]}
        for c in range(NCORES)
    ]
    res = run_bass_kernel_spmd(nc, in_maps, list(range(NCORES)), trace=trace)
    val = np.float32(res.results[0]["out"][0, 0])
    return val, res


def kernel(input, target):
    val, _ = _run(input, target)
    return np.float32(val).reshape(())


# revision 29
# speedup vs baseline: 1.0097x; 1.0097x over previous
"""DRAE loss kernel for Trainium2, 8 NeuronCores (SPMD).

Problem: input/target [8192, 4096] f32.
  Err[n] = sum_d (input[n,d] - target[n,d])^2            (memory-bound part)
  sErr = sort(Err); cs = cumsum(sErr)
  obj(k) = (total2 - cs_k^2/k - (total-cs_k)^2/(N-k)) / Sb   for k=1..N-1
  i = argmin(obj)  (first min);  out = cs[i]/(i+1) + 0.1*obj[i]

Sharding: data-parallel over N across 8 cores (1024 rows each). Per-core
row reductions produce Err_local[1024]; AllGather -> Err[8192]; the sort +
cumsum + argmin + final scalar run replicated on every core on-chip
(exact bitonic sort network on a [128,64]/[64,128] two-layout scheme).

Self-contained: hardcodes shapes; only needs concourse (bass) + numpy.
"""
import numpy as np

import concourse.bass as bass
import concourse.bacc as bacc
import concourse.mybir as mybir
import concourse.tile as tile
from concourse.bass_utils import run_bass_kernel_spmd

F32 = mybir.dt.float32
I32 = mybir.dt.int32

NCORES = 8
N, D = 8192, 4096
ROWS = N // NCORES           # 1024 rows per core
RT = ROWS // 128             # 8 row-tiles of [128, D] per core
W_A, W_B = 64, 128           # layout A: [128, 64]; layout B: [64, 128]
LAMB = 0.1
BIG = np.float32(1e30)

_CACHE = {}


def _emit_sort(nc, ps, ta, tb, tflip, ident):
    """Normalized bitonic sort network (reversal substages, all-ascending).

    ta = [ta0, ta1] ping-pong [128,64] SBUF tiles (layout A: i = 64p + f),
    tb = [tb0, tb1] [64,128] tiles (layout B: i = 64r + q, transpose of A),
    tflip = [128,64] scratch for the free-reversed copy feeding pb_rev,
    ident = [128,128] f32 identity for PE transposes.
    Data starts in ta[0]; returns index of the ta tile holding sorted data.
    """
    mm = mybir.AluOpType
    ia = 0
    ib = 0

    def plain(x_ap, y_ap, b):
        # compare-exchange pairs at stride b in the free dim; lower keeps min
        xv = x_ap.rearrange("p (a t b) -> p a t b", t=2, b=b)
        yv = y_ap.rearrange("p (a t b) -> p a t b", t=2, b=b)
        nc.vector.tensor_tensor(yv[:, :, 0, :], xv[:, :, 0, :], xv[:, :, 1, :], mm.min)
        nc.vector.tensor_tensor(yv[:, :, 1, :], xv[:, :, 0, :], xv[:, :, 1, :], mm.max)

    for s in range(1, 14):
        if s <= 6:
            # reversal substage within the free dim of A
            blk = 1 << s
            h = blk // 2
            x, y = ta[ia][:], ta[1 - ia][:]
            xv = x.rearrange("p (a b) -> p a b", b=blk)
            xr = xv[:, :, ::-1]
            yv = y.rearrange("p (a b) -> p a b", b=blk)
            nc.vector.tensor_tensor(yv[:, :, :h], xv[:, :, :h], xr[:, :, :h], mm.min)
            nc.vector.tensor_tensor(yv[:, :, h:], xv[:, :, h:], xr[:, :, h:], mm.max)
            ia = 1 - ia
            for j in range(s - 2, -1, -1):
                plain(ta[ia][:], ta[1 - ia][:], 1 << j)
                ia = 1 - ia
        else:
            # A -> B with both plain and partition-reversed copies
            nc.vector.tensor_copy(tflip[:], ta[ia][:][:, ::-1])
            pb = ps.tile([64, 128], F32, tag="pb", name="pb")
            nc.tensor.transpose(pb[:], ta[ia][:], ident[:])
            pbr = ps.tile([64, 128], F32, tag="pbr", name="pbr")
            nc.tensor.transpose(pbr[:], tflip[:], ident[:])
            # only one DVE input may come from PSUM: land pb in SBUF first
            nc.vector.tensor_copy(tb[ib][:], pb[:])
            # reversal substage in B: partner(q, r) = (63-q, blockrev(r))
            rblk = 1 << (s - 6)
            h = rblk // 2
            vb = tb[ib][:].rearrange("q (a b) -> q a b", b=rblk)
            vr = pbr[:].rearrange("q (a b) -> q a b", b=rblk)[:, :, ::-1]
            yv = tb[1 - ib][:].rearrange("q (a b) -> q a b", b=rblk)
            nc.vector.tensor_tensor(yv[:, :, :h], vb[:, :, :h], vr[:, :, :h], mm.min)
            nc.vector.tensor_tensor(yv[:, :, h:], vb[:, :, h:], vr[:, :, h:], mm.max)
            ib = 1 - ib
            for j in range(s - 2, 5, -1):
                plain(tb[ib][:], tb[1 - ib][:], 1 << (j - 6))
                ib = 1 - ib
            # B -> A, landing in SBUF before the A substages
            pa = ps.tile([128, 64], F32, tag="psA", name="pa")
            nc.tensor.transpose(pa[:], tb[ib][:], ident[:64, :64])
            nc.vector.tensor_copy(ta[1 - ia][:], pa[:])
            ia = 1 - ia
            for j in range(5, -1, -1):
                plain(ta[ia][:], ta[1 - ia][:], 1 << j)
                ia = 1 - ia
    return ia


def _build(phase2_only=False, stop="full", timing_variant=False):
    ncores = 1 if (phase2_only or timing_variant) else NCORES
    nc = bacc.Bacc("TRN2", target_bir_lowering=False, debug=False,
                   num_devices=ncores)

    if phase2_only:
        err_in = nc.dram_tensor("err", [N], F32, kind="ExternalInput").ap()
        dbg_srt = nc.dram_tensor("dbg_srt", [128, W_A], F32, kind="ExternalOutput").ap()
        dbg_cs = nc.dram_tensor("dbg_cs", [128, W_A], F32, kind="ExternalOutput").ap()
        dbg_obj = nc.dram_tensor("dbg_obj", [128, W_A], F32, kind="ExternalOutput").ap()
    else:
        inp = nc.dram_tensor("input", [ROWS, D], F32, kind="ExternalInput").ap()
        tgt = nc.dram_tensor("target", [ROWS, D], F32, kind="ExternalInput").ap()
    out = nc.dram_tensor("out", [1, 1], F32, kind="ExternalOutput").ap()

    # compile-time constants
    kvals = (np.arange(1, N + 1, dtype=np.float64).reshape(128, 64)).astype(np.float32)
    recip_k = (1.0 / kvals).astype(np.float32)
    nk = (N - kvals).astype(np.float32)
    nk[127, 63] = 1.0  # k = N slot excluded later; avoid 1/0
    recip_nk = (1.0 / nk).astype(np.float32)
    recip_nk[127, 63] = 0.0
    ident_np = np.eye(128, dtype=np.float32)
    triu_np = np.triu(np.ones((128, 128), np.float32), 1)  # [p',p]=1 iff p'<p
    ones_np = np.ones((128, 128), np.float32)
    excl_np = np.zeros((128, 64), np.float32)
    excl_np[127, 63] = BIG  # push k=N out of the argmin

    c_k = nc.inline_tensor(kvals, name="c_k")
    c_rk = nc.inline_tensor(recip_k, name="c_rk")
    c_rnk = nc.inline_tensor(recip_nk, name="c_rnk")
    c_id = nc.inline_tensor(ident_np, name="c_id")
    c_tu = nc.inline_tensor(triu_np, name="c_tu")
    c_on = nc.inline_tensor(ones_np, name="c_on")
    c_ex = nc.inline_tensor(excl_np, name="c_ex")

    mm = mybir.AluOpType
    AF = mybir.ActivationFunctionType

    with tile.TileContext(nc) as tc:
        with (
            tc.tile_pool(name="io", bufs=3) as io,
            tc.tile_pool(name="wk", bufs=2) as wk,
            tc.tile_pool(name="st", bufs=1) as st,
            tc.tile_pool(name="ps", bufs=2, space="PSUM") as pspool,
            tc.tile_pool(name="dram", bufs=1, space="DRAM") as dram,
        ):
            def _body():
                if not phase2_only:
                    # ---------------- phase 1: Err_local ----------------
                    errcol = st.tile([128, RT], F32, name="errcol")
                    for t in range(RT):
                        a = io.tile([128, D], F32, tag="a", name="a")
                        b = io.tile([128, D], F32, tag="b", name="b")
                        nc.sync.dma_start(a[:], inp[t * 128:(t + 1) * 128, :])
                        nc.sync.dma_start(b[:], tgt[t * 128:(t + 1) * 128, :])
                        d = wk.tile([128, D], F32, tag="d", name="d")
                        nc.vector.tensor_tensor(d[:], a[:], b[:], mm.subtract)
                        sq = wk.tile([128, D], F32, tag="sq", name="sq")
                        nc.scalar.activation(sq[:], d[:], AF.Square,
                                             accum_out=errcol[:, t:t + 1])

                    # ---------------- allgather Err ----------------
                    gin = dram.tile([ROWS], F32, name="gin")
                    gout = dram.tile([N], F32, name="gout")
                    nc.sync.dma_start(gin[:].rearrange("(p t) -> p t", t=RT),
                                      errcol[:])
                    if timing_variant:
                        # stand-in for the AllGather: 8 local 4KB DMAs
                        for c in range(NCORES):
                            nc.sync.dma_start(gout[c * ROWS:(c + 1) * ROWS],
                                              gin[:])
                    else:
                        nc.gpsimd.collective_compute(
                            "AllGather", mm.bypass,
                            replica_groups=[list(range(NCORES))],
                            ins=[gin[:]], outs=[gout[:]],
                        )
                    err_src = gout[:]
                    if stop == "phase1":
                        nc.sync.dma_start(out[:], errcol[:1, :1])
                        return
                else:
                    err_src = err_in

                # ---------------- phase 2 (replicated) ----------------
                ta = [st.tile([128, W_A], F32, tag=f"ta{i}", name=f"ta{i}")
                      for i in range(2)]
                tb = [st.tile([64, W_B], F32, tag=f"tb{i}", name=f"tb{i}")
                      for i in range(2)]
                tflip = st.tile([128, W_A], F32, name="tflip")
                ident = st.tile([128, 128], F32, name="ident")
                triu = st.tile([128, 128], F32, name="triu")
                ones = st.tile([128, 128], F32, name="ones")
                kf = st.tile([128, W_A], F32, name="kf")
                rk = st.tile([128, W_A], F32, name="rk")
                rnk = st.tile([128, W_A], F32, name="rnk")
                excl = st.tile([128, W_A], F32, name="excl")
                for tl, cc in ((ident, c_id), (triu, c_tu),
                               (ones, c_on), (kf, c_k), (rk, c_rk),
                               (rnk, c_rnk), (excl, c_ex)):
                    nc.sync.dma_start(tl[:], cc.ap())

                nc.sync.dma_start(ta[0][:],
                                  err_src.rearrange("(p f) -> p f", f=W_A))

                isorted = _emit_sort(nc, pspool, ta, tb, tflip, ident)
                srt = ta[isorted][:]      # sorted ascending, [p,f] = s[64p+f]

                if phase2_only and stop == "sort":
                    nc.sync.dma_start(dbg_srt[:], srt)
                    nc.sync.dma_start(out[:], srt[:1, :1])
                    return

                # row sums of squares -> total2
                sqd = st.tile([128, W_A], F32, name="sqd")
                rowsq = st.tile([128, 1], F32, name="rowsq")
                nc.scalar.activation(sqd[:], srt, AF.Square, accum_out=rowsq[:])

                # in-row inclusive prefix sums (native DVE scan)
                cs = [st.tile([128, W_A], F32, tag=f"cs{i}", name=f"cs{i}")
                      for i in range(2)]
                nc.vector.tensor_tensor_scan(cs[0][:], srt, srt, 0.0,
                                             mm.add, mm.bypass)
                rowpref = cs[0]           # [128,64] within-row inclusive prefix

                # partition-level exclusive prefix + totals via PE
                rowtot = rowpref[:, W_A - 1:W_A]
                pexc = pspool.tile([128, 1], F32, tag="psv", name="pexc")
                nc.tensor.matmul(pexc[:], triu[:], rowtot)
                ptot = pspool.tile([128, 1], F32, tag="psv", name="ptot")
                nc.tensor.matmul(ptot[:], ones[:], rowtot)
                ptot2 = pspool.tile([128, 1], F32, tag="psv", name="ptot2")
                nc.tensor.matmul(ptot2[:], ones[:], rowsq[:])
                exc = st.tile([128, 1], F32, name="exc")
                tot = st.tile([128, 1], F32, name="tot")
                tot2 = st.tile([128, 1], F32, name="tot2")
                nc.vector.tensor_copy(exc[:], pexc[:])
                nc.vector.tensor_copy(tot[:], ptot[:])
                nc.vector.tensor_copy(tot2[:], ptot2[:])

                csf = cs[1]
                nc.vector.tensor_scalar(csf[:], rowpref[:], exc[:], None, mm.add)

                if phase2_only and stop == "cs":
                    nc.sync.dma_start(dbg_cs[:], csf[:])
                    nc.sync.dma_start(out[:], csf[:1, :1])
                    return

                # obj = (total2 - cs^2/k - (tot-cs)^2/(N-k)) / Sb, computed as
                # v/negSb with v = w - total2, negSb = tot*allMean - total2
                t1 = st.tile([128, W_A], F32, tag="t1", name="t1")
                nc.vector.tensor_tensor(t1[:], csf[:], csf[:], mm.mult)
                nc.vector.tensor_tensor(t1[:], t1[:], rk[:], mm.mult)
                u = st.tile([128, W_A], F32, tag="u", name="u")
                nc.vector.tensor_scalar(u[:], csf[:], tot[:], None, mm.subtract)
                nc.vector.tensor_tensor(u[:], u[:], u[:], mm.mult)
                nc.vector.tensor_tensor(u[:], u[:], rnk[:], mm.mult)
                obj = st.tile([128, W_A], F32, tag="obj", name="obj")
                nc.vector.tensor_tensor(obj[:], t1[:], u[:], mm.add)
                nc.vector.tensor_scalar(obj[:], obj[:], tot2[:], None, mm.subtract)

                am = st.tile([128, 1], F32, name="am")   # allMean
                nc.vector.tensor_scalar(am[:], tot[:], float(1.0 / N), None, mm.mult)
                nsb = st.tile([128, 1], F32, name="nsb")  # negSb
                nc.vector.tensor_tensor(nsb[:], tot[:], am[:], mm.mult)
                nc.vector.tensor_tensor(nsb[:], nsb[:], tot2[:], mm.subtract)
                rnsb = st.tile([128, 1], F32, name="rnsb")
                nc.vector.reciprocal(rnsb[:], nsb[:])
                nc.vector.tensor_scalar(obj[:], obj[:], rnsb[:], None, mm.mult)

                # exclude k = N (BIG at the last slot, 0 elsewhere)
                nc.vector.tensor_tensor(obj[:], obj[:], excl[:], mm.add)

                if phase2_only and stop == "obj":
                    nc.sync.dma_start(dbg_obj[:], obj[:])
                    nc.sync.dma_start(out[:], obj[:1, :1])
                    return

                # argmin (first-min): gmin, then smallest k with obj==gmin
                rmin = st.tile([128, 1], F32, name="rmin")
                nc.vector.tensor_reduce(rmin[:], obj[:], mybir.AxisListType.X, mm.min)
                prm = pspool.tile([1, 128], F32, tag="psv", name="prm")
                nc.tensor.transpose(prm[:], rmin[:], ident[:])
                gmin = st.tile([1, 1], F32, name="gmin")
                nc.vector.tensor_reduce(gmin[:], prm[:], mybir.AxisListType.X, mm.min)
                pgm = pspool.tile([128, 1], F32, tag="psv", name="pgm")
                nc.tensor.matmul(pgm[:], ones[:1, :], gmin[:])
                gminb = st.tile([128, 1], F32, name="gminb")
                nc.vector.tensor_copy(gminb[:], pgm[:])

                eq = st.tile([128, W_A], I32, tag="eq", name="eq")
                nc.vector.tensor_scalar(eq[:], obj[:], gminb[:], None, mm.is_equal)
                idxv = st.tile([128, W_A], F32, tag="idxv", name="idxv")
                nc.vector.memset(idxv[:], float(BIG))
                nc.vector.copy_predicated(idxv[:], eq[:], kf[:])
                ridx = st.tile([128, 1], F32, name="ridx")
                nc.vector.tensor_reduce(ridx[:], idxv[:], mybir.AxisListType.X, mm.min)
                pri = pspool.tile([1, 128], F32, tag="psv", name="pri")
                nc.tensor.transpose(pri[:], ridx[:], ident[:])
                gidx = st.tile([1, 1], F32, name="gidx")
                nc.vector.tensor_reduce(gidx[:], pri[:], mybir.AxisListType.X, mm.min)
                pgi = pspool.tile([128, 1], F32, tag="psv", name="pgi")
                nc.tensor.matmul(pgi[:], ones[:1, :], gidx[:])
                gidxb = st.tile([128, 1], F32, name="gidxb")
                nc.vector.tensor_copy(gidxb[:], pgi[:])

                if phase2_only and stop == "argmin":
                    nc.sync.dma_start(out[:], gidx[:])
                    return

                # cs[i*] via one-hot dot
                oh = st.tile([128, W_A], F32, tag="oh", name="oh")
                nc.vector.tensor_scalar(oh[:], kf[:], gidxb[:], None, mm.is_equal)
                dump = st.tile([128, W_A], F32, tag="dump", name="dump")
                csrow = st.tile([128, 1], F32, name="csrow")
                nc.vector.tensor_tensor(dump[:], csf[:], oh[:], mm.mult)
                nc.vector.tensor_reduce(csrow[:], dump[:], mybir.AxisListType.X,
                                        mm.add)
                pcr = pspool.tile([1, 128], F32, tag="psv", name="pcr")
                nc.tensor.transpose(pcr[:], csrow[:], ident[:])
                cssum = st.tile([1, 1], F32, name="cssum")
                nc.vector.tensor_reduce(cssum[:], pcr[:], mybir.AxisListType.X, mm.add)

                # out = cssum/T + 0.1*gmin
                rT = st.tile([1, 1], F32, name="rT")
                nc.vector.reciprocal(rT[:], gidx[:])
                res = st.tile([1, 1], F32, name="res")
                nc.vector.tensor_tensor(res[:], cssum[:], rT[:], mm.mult)
                sg = st.tile([1, 1], F32, name="sg")
                nc.vector.tensor_scalar(sg[:], gmin[:], LAMB, None, mm.mult)
                nc.vector.tensor_tensor(res[:], res[:], sg[:], mm.add)
                nc.sync.dma_start(out[:], res[:])

                if phase2_only:
                    nc.sync.dma_start(dbg_srt[:], srt)
                    nc.sync.dma_start(dbg_cs[:], csf[:])
                    nc.sync.dma_start(dbg_obj[:], obj[:])

            _body()

    nc.compile()
    return nc


def _get_program():
    if "nc" not in _CACHE:
        _CACHE["nc"] = _build()
    return _CACHE["nc"]


def _run(input, target, trace=False):
    nc = _get_program()
    input = np.ascontiguousarray(input, dtype=np.float32)
    target = np.ascontiguousarray(target, dtype=np.float32)
    assert input.shape == (N, D) and target.shape == (N, D)
    in_maps = [
        {"input": input[c * ROWS:(c + 1) * ROWS],
         "target": target[c * ROWS:(c + 1) * ROWS]}
        for c in range(NCORES)
    ]
    res = run_bass_kernel_spmd(nc, in_maps, list(range(NCORES)), trace=trace)
    val = np.float32(res.results[0]["out"][0, 0])
    return val, res


def kernel(input, target):
    val, _ = _run(input, target)
    return np.float32(val).reshape(())
